# revision 23
# baseline (speedup 1.0000x reference)
"""Bahdanau attention kernel for Trainium2 (Bass/Tile), data-parallel over batch
with the W linear layer sharded across cores (AllToAll energy exchange).

Problem (full shapes):
    encoder_output   [S=2048, B=16, H=1024] f32
    last_decoder_state [2, 1, B, H] f32   (only [0,0] used -> state [B, H])
    W [H, H], b [H]
    energy = state @ W.T + b                  [B, H]  (nn.Linear)
    scores = einsum('sbh,bh->sb', enc, energy) [S, B]
    out    = softmax(scores, axis=0)[None, None]  [1, 1, S, B]

The kernel is DMA-bound (memory regime).  enc is cast to fp16 on the host
(validated: rel err 4.5e-3 vs the 2e-2 gate; bf16 fails) and batch-split
8 ways: 8.39 MB/core — an exact 1/8 of the tensor, the irreducible
per-core traffic.  The measured per-NC HBM ceiling here is ~358 GB/s, and
the previous kernel hit 100% of it, so the only lever left was traffic:
it also streamed a replicated 2 MB fp16 W per core.  This version shards
W row-wise instead — each core loads W[c*128:(c+1)*128, :] (256 KB),
computes energy[jslice_c, all 16 batches], and an AllToAll (4 KB/core)
hands every core the full-H energy for exactly its own 2 batches at a
rank-independent layout (core c sends chunk d = batches of core d; SPMD
cores can then all read "my batches" at the same local offsets, which a
shared program + AllGather could not express).  Per-core traffic drops
10.65 -> 8.72 MB.

Per-core device program (host-side layouts are DMA-natural):
    esT[b, j]  = sum_i state[b,i] W[jsl+j,i] (+bias)   PE: state stationary,
        W slice moving (1k cycles); bias via DVE add in f32 -> fp16
    AllToAll esT -> rcv[g*2+bl, j] = energy[2c+bl, g*128+j]  (4 KB)
    one PE transpose (16-identity) -> energy[j, jt, bl] fp16 in PSUM,
        scalar copy to SBUF
    scores[b, s] = sum_h energy[h, b] enc[b, h, s]     (PE, fp16, f32 PSUM
        accum; the two batches run concurrently in separate 32-col groups
        via tile_position derived from out partitions 0/32)
    probs = softmax over s (joint [33,*] ops: both batches per instruction)

Ring assignment (all measured): 6 of the 8 enc MB-sized transfers ride
the sync HWDGE ring and 2 the scalar HWDGE ring (one ring alone caps at
~337 GB/s; split, the pure stream hits the ~358 GB/s per-NC HBM
ceiling).  Every other data DMA also rides a HWDGE ring — one routine
SWDGE (gpsimd) load measured +3.2 us/rep of Q7 descriptor-emission cost.
Only the collective itself is issued from gpsimd.  The softmax + output
stores of rep N are emitted AFTER rep N+1's input loads (software
pipelining): the ACT sequencer both executes Exp and issues the
scalar-ring DMAs, so late-dependency work must sit behind the next rep's
loads or it stalls the input stream.  Scores accumulate into 4 per-chunk
PSUM banks drained to SBUF by DVE as each chunk completes.

Known residual: the AllToAll's serial latency is ~5.7 us idle but
~23 us under the full enc stream (SDMA packet round-robin + loaded-HBM
handshake costs), and NRT executes collectives strictly serially, so it
costs ~3 us/rep on top of the 24.5 us DMA floor (measured via a local
loopback A/B).  Shared-output AllGather, sub-mesh groups, gpsimd bounce
placement, and deeper pipelining were all measured and did not help.

`reps` exists only for benchmarking: the body is repeated (statically
unrolled — collectives cannot live inside a For_i hardware loop, and the
loop barrier costs ~3.4 us/rep anyway) inside one NEFF so steady-state HW
time per rep can be measured through the high-latency axon dispatch path.
Every rep is a self-contained emulation of one kernel() call: it re-loads
all inputs from DRAM and re-runs the full computation.  kernel() always
uses reps=1.

Measured (median-of-9 wall clock, marginal 64->1024 reps): ~28.5 us/rep
vs 33.4 us/rep for the replicated-W baseline under the same harness;
floor is ~24.5 us (enc stream at the HBM ceiling) + ~3 us collective.
"""

import numpy as np

S, B, H = 2048, 16, 1024
NCORES = 8
BL = B // NCORES  # 2 batches per core
P = 128           # partitions
HT = H // P       # 8 h-tiles
SCW = 512         # matmul moving-operand chunk (one PSUM bank of f32)
SC = S // SCW     # 4 seq chunks
ENC_BUFS = 12     # 12 MB of enc runway so the input stream never waits on
                  # the (energy-chain-gated) scores consumer
ENC_SCALAR = 2    # how many of the 8 enc h-tiles ride the scalar ring
SIM_PSUM_INIT = False  # CoreSim only: memset score PSUM so partitions 1-31
                       # (never written on HW, never read downstream) don't
                       # trip the uninitialized-memory check
NO_CC = False          # timing probe only: local loopback instead of AllToAll
CC_SINGLETON = False   # timing probe only: per-core singleton replica groups
CC_ALLGATHER_STATIC = False  # timing probe only: Shared AllGather + WRONG static row read
                       # (outputs wrong for cores != contribution layout)
CONSTS_BUFS = 2
DRAM_BUFS = 2
WG = 8             # W-shard group size (sub-mesh replica groups are not
                   # supported by NRT here, so this must stay 8)

_cached = {}


def _build_nc(reps=1):
    import concourse.bacc as bacc
    import concourse.bass as bass
    import concourse.tile as tile
    from concourse import mybir

    f16 = mybir.dt.float16
    f32 = mybir.dt.float32
    nc = bacc.Bacc("TRN2", target_bir_lowering=False, debug=False, num_devices=NCORES)

    JW = H // WG       # W rows (energy features) per core
    SUBT = JW // P     # 128-wide sub-tiles per slice
    GB = BL * WG       # batches per replica group

    # host-prepped layouts (see prep_in_maps), c = core id, r = c % WG,
    # g = c // WG:
    # enc_t[ht, p, bl, s] = enc[s, 2c+bl, ht*128+p]           fp16
    # wslt[p, it, jj]     = W[r*JW + jj, it*128 + p]          fp16
    # stT[p, it, k]       = state[g*GB + k, it*128 + p]       fp16
    # bias_bc[k, jj]      = bias[r*JW + jj]                   f32
    # id16                = eye(16)                           fp16
    enc_t = nc.dram_tensor("enc_t", [HT, P, BL, S], f16, kind="ExternalInput").ap()
    wslt = nc.dram_tensor("wslt", [P, HT, JW], f16, kind="ExternalInput").ap()
    stT = nc.dram_tensor("stT", [P, HT, GB], f16, kind="ExternalInput").ap()
    bias_bc = nc.dram_tensor("bias_bc", [GB, JW], f32, kind="ExternalInput").ap()
    id16 = nc.dram_tensor("id16", [B, B], f16, kind="ExternalInput").ap()
    probs = nc.dram_tensor("probs", [BL, S], f32, kind="ExternalOutput").ap()

    with tile.TileContext(nc) as tc:
        with (
            tc.tile_pool(name="consts", bufs=CONSTS_BUFS) as consts,
            tc.tile_pool(name="wpool", bufs=2) as wpool,
            tc.tile_pool(name="encpool", bufs=ENC_BUFS) as encpool,
            tc.tile_pool(name="dram", bufs=DRAM_BUFS, space="DRAM") as dram,
            tc.tile_pool(name="e_ps", bufs=2, space=bass.MemorySpace.PSUM) as e_pool,
            tc.tile_pool(name="t_ps", bufs=2, space=bass.MemorySpace.PSUM) as t_pool,
            tc.tile_pool(name="sc_ps", bufs=1, space=bass.MemorySpace.PSUM) as ps_pool,
            tc.tile_pool(name="spool", bufs=2) as spool,
        ):

            def emit_head():
                """Loads + energy chain + scores of one rep.  Returns the
                state the deferred softmax tail needs."""
                # --- input loads, all on HWDGE rings (a single SWDGE
                # gpsimd load measured +3.2 us/rep; Q7 descriptor emission
                # is far more expensive than the bytes)
                st = consts.tile([P, HT, GB], f16)
                nc.scalar.dma_start(out=st[:], in_=stT)
                ws = wpool.tile([P, HT, JW], f16)
                nc.scalar.dma_start(out=ws[:], in_=wslt)
                bb = consts.tile([GB, JW], f32)
                nc.scalar.dma_start(out=bb[:], in_=bias_bc)
                idt = consts.tile([B, B], f16)
                nc.scalar.dma_start(out=idt[:], in_=id16)

                ets = [None] * HT
                for ht in range(HT):
                    et = encpool.tile([P, BL, S], f16)
                    eng = nc.scalar if ht < ENC_SCALAR else nc.sync
                    eng.dma_start(out=et[:], in_=enc_t[ht])
                    ets[ht] = et

                # --- energy slice: esT[k, jj] for the group's batches
                esT_ps = e_pool.tile([GB, JW], f32)
                for it in range(HT):
                    nc.tensor.matmul(
                        esT_ps[:],
                        st[:, it, :],    # lhsT [i, b] (stationary)
                        ws[:, it, :],    # rhs  [i, j] (moving)
                        start=(it == 0),
                        stop=(it == HT - 1),
                    )
                esT = consts.tile([GB, JW], f16)
                nc.vector.tensor_tensor(
                    out=esT[:], in0=esT_ps[:], in1=bb[:], op=mybir.AluOpType.add
                )

                # --- AllToAll: chunk d (batches 2d:2d+2) of my j-slice ->
                # core d; only the collective itself runs on gpsimd — every
                # SWDGE dma_start measured ~1-3 us of Q7 descriptor-emission
                # cost, so all data DMAs ride the HWDGE rings
                # ib row 2d+bl = chunk d (batches 2d:2d+2 of my j-slice)
                ib = dram.tile([B, P], f16)
                nc.scalar.dma_start(out=ib[:], in_=esT[:])
                rcv = dram.tile([B, P], f16)
                if NO_CC:
                    nc.scalar.dma_start(out=rcv[:], in_=ib[:])
                    g_src = rcv[:]
                elif CC_ALLGATHER_STATIC:
                    rcv_g = dram.tile([NCORES, B, P], f16, addr_space="Shared")
                    nc.gpsimd.collective_compute(
                        "AllGather",
                        mybir.AluOpType.bypass,
                        replica_groups=[list(range(NCORES))],
                        ins=[ib.opt()],
                        outs=[rcv_g.opt()],
                    )
                    # WRONG rows (0:2 instead of 2c:2c+2) - timing only
                    g_src = rcv_g[:, 0:BL, :]
                else:
                    groups = ([[c] for c in range(NCORES)] if CC_SINGLETON
                              else [list(range(g * WG, (g + 1) * WG))
                                    for g in range(NCORES // WG)])
                    nc.gpsimd.collective_compute(
                        "AllToAll",
                        mybir.AluOpType.bypass,
                        replica_groups=groups,
                        ins=[ib.opt()],
                        outs=[rcv.opt()],
                    )
                    g_src = rcv[:]
                # rcv[src*BL+bl, jj] = energy[my batch bl, src*128+jj]
                g_sb = consts.tile([B, P], f16)
                nc.scalar.dma_start(out=g_sb[:], in_=g_src)

                # --- transpose to energy[j, (jt, bl)] for the scores lhsT
                gT_ps = t_pool.tile([P, B], f16)
                nc.tensor.transpose(gT_ps[:], g_sb[:], idt[:])
                esb = consts.tile([P, HT, BL], f16)
                nc.scalar.activation(
                    out=esb[:], in_=gT_ps[:],
                    func=mybir.ActivationFunctionType.Identity,
                    bias=0.0, scale=1.0,
                )

                # --- scores psum: b=0 on partition 0, b=1 on partition 32;
                # the two batches run concurrently in separate 32-col
                # groups.  One PSUM bank ([33, SCW]) per seq chunk; DVE
                # drains each chunk to SBUF (and takes its max) as soon as
                # it completes so the banks recycle without waiting for the
                # softmax, which is emitted a rep later (see emit_tail).
                pscs = [
                    ps_pool.tile([33, SCW], f32, name=f"psc{sc}")
                    for sc in range(SC)
                ]
                if SIM_PSUM_INIT:
                    for psc in pscs:
                        nc.vector.memset(psc[:], 0.0)
                sc_sb = spool.tile([33, S], f32)
                nmaxs = spool.tile([33, SC], f32)
                for ht in range(HT):
                    et = ets[ht]
                    for sc in range(SC):
                        for b in range(BL):
                            nc.tensor.matmul(
                                pscs[sc][32 * b:32 * b + 1, :],
                                esb[:, ht, b:b + 1],                  # lhsT [h, 1]
                                et[:, b, sc * SCW:(sc + 1) * SCW],    # rhs [h, s]
                                start=(ht == 0),
                                stop=(ht == HT - 1),
                                tile_position=(0, 32 * b),
                            )
                        if ht == HT - 1:
                            nc.vector.reduce_max(
                                nmaxs[:, sc:sc + 1], pscs[sc][:],
                                axis=mybir.AxisListType.X, negate=True,
                            )
                            nc.vector.tensor_copy(
                                out=sc_sb[:, sc * SCW:(sc + 1) * SCW],
                                in_=pscs[sc][:],
                            )
                return sc_sb, nmaxs

            def emit_tail(sc_sb, nmaxs):
                """Softmax + store of the PREVIOUS rep.  Emitted after the
                next rep's loads so the in-order ACT sequencer (which both
                executes Exp and issues the scalar-ring DMAs) never makes
                the input stream wait on late compute."""
                prob_sb = spool.tile([33, S], f32)
                nmax = spool.tile([33, 1], f32)
                ssums = spool.tile([33, SC], f32)
                ssum = spool.tile([33, 1], f32)
                rinv = spool.tile([33, 1], f32)
                nc.vector.tensor_reduce(
                    out=nmax[:], in_=nmaxs[:], op=mybir.AluOpType.min,
                    axis=mybir.AxisListType.X,
                )
                for sc in range(SC):
                    nc.scalar.activation(
                        out=prob_sb[:, sc * SCW:(sc + 1) * SCW],
                        in_=sc_sb[:, sc * SCW:(sc + 1) * SCW],
                        func=mybir.ActivationFunctionType.Exp,
                        bias=nmax[:],
                        scale=1.0,
                        accum_out=ssums[:, sc:sc + 1],
                    )
                nc.vector.tensor_reduce(
                    out=ssum[:], in_=ssums[:], op=mybir.AluOpType.add,
                    axis=mybir.AxisListType.X,
                )
                nc.vector.reciprocal(rinv[:], ssum[:])
                nc.vector.tensor_scalar_mul(
                    out=prob_sb[:], in0=prob_sb[:], scalar1=rinv[:]
                )
                for b in range(BL):
                    nc.scalar.dma_start(
                        out=probs[b:b + 1, :], in_=prob_sb[32 * b:32 * b + 1, :]
                    )

            pending = None
            for _rep in range(reps):
                state = emit_head()
                if pending is not None:
                    emit_tail(*pending)
                pending = state
            emit_tail(*pending)

    nc.compile()
    return nc


def get_nc(reps=1, dynamic=False):
    # `dynamic` is accepted for interface compat but ignored: collectives
    # cannot live inside a For_i hardware loop, so reps are always
    # statically unrolled.
    key = ("nc", reps, ENC_BUFS, ENC_SCALAR, SIM_PSUM_INIT, NO_CC,
           CC_SINGLETON, CC_ALLGATHER_STATIC, CONSTS_BUFS, DRAM_BUFS, WG)
    if key not in _cached:
        _cached[key] = _build_nc(reps)
    return _cached[key]


def prep_in_maps(encoder_output, last_decoder_state, W, b):
    JW = H // WG
    GB = BL * WG
    enc16 = np.asarray(encoder_output, dtype=np.float32).astype(np.float16)  # [S,B,H]
    state = np.asarray(last_decoder_state, dtype=np.float32)[0, 0]           # [B,H]
    W16 = np.asarray(W, dtype=np.float32).astype(np.float16)
    bias = np.asarray(b, dtype=np.float32)
    st16 = state.astype(np.float16)
    id16 = np.eye(B, dtype=np.float16)
    in_maps = []
    for c in range(NCORES):
        r, g = c % WG, c // WG
        b0 = BL * c
        ec = enc16[:, b0:b0 + BL, :]                                         # [S,BL,H]
        enc_t = np.ascontiguousarray(ec.transpose(2, 1, 0)).reshape(HT, P, BL, S)
        wsl = W16[r * JW:(r + 1) * JW, :]                                    # [jj, H]
        wslt = np.ascontiguousarray(wsl.reshape(JW, HT, P).transpose(2, 1, 0))
        stT = np.ascontiguousarray(
            st16[g * GB:(g + 1) * GB].reshape(GB, HT, P).transpose(2, 1, 0)
        )  # [p, it, k]
        bias_bc = np.ascontiguousarray(
            np.broadcast_to(bias[r * JW:(r + 1) * JW], (GB, JW))
        )
        in_maps.append({"enc_t": enc_t, "wslt": wslt, "stT": stT,
                        "bias_bc": bias_bc, "id16": id16})
    return in_maps


def assemble(results):
    out = np.empty((S, B), np.float32)
    for c in range(NCORES):
        out[:, BL * c:BL * (c + 1)] = results[c]["probs"].T
    return out[None, None]


def kernel(encoder_output, last_decoder_state, W, b):
    from concourse.bass_utils import run_bass_kernel_spmd

    nc = get_nc()
    in_maps = prep_in_maps(encoder_output, last_decoder_state, W, b)
    res = run_bass_kernel_spmd(nc, in_maps, core_ids=list(range(NCORES)))
    return assemble(res.results)


# revision 24
# speedup vs baseline: 1.0707x; 1.0707x over previous
"""Bahdanau attention kernel for Trainium2 (Bass/Tile), data-parallel over batch
with the W linear layer sharded across cores (AllToAll energy exchange).

Problem (full shapes):
    encoder_output   [S=2048, B=16, H=1024] f32
    last_decoder_state [2, 1, B, H] f32   (only [0,0] used -> state [B, H])
    W [H, H], b [H]
    energy = state @ W.T + b                  [B, H]  (nn.Linear)
    scores = einsum('sbh,bh->sb', enc, energy) [S, B]
    out    = softmax(scores, axis=0)[None, None]  [1, 1, S, B]

The kernel is DMA-bound (memory regime).  enc is cast to fp16 on the host
(validated: rel err 4.5e-3 vs the 2e-2 gate; bf16 fails) and batch-split
8 ways: 8.39 MB/core — an exact 1/8 of the tensor, the irreducible
per-core traffic.  The measured per-NC HBM ceiling here is ~358 GB/s, and
the previous kernel hit 100% of it, so the only lever left was traffic:
it also streamed a replicated 2 MB fp16 W per core.  This version shards
W row-wise instead — each core loads W[c*128:(c+1)*128, :] (256 KB),
computes energy[jslice_c, all 16 batches], and an AllToAll (4 KB/core)
hands every core the full-H energy for exactly its own 2 batches at a
rank-independent layout (core c sends chunk d = batches of core d; SPMD
cores can then all read "my batches" at the same local offsets, which a
shared program + AllGather could not express).  Per-core traffic drops
10.65 -> 8.72 MB.

Per-core device program (host-side layouts are DMA-natural):
    esT[b, j]  = sum_i state[b,i] W[jsl+j,i] (+bias)   PE: state stationary,
        W slice moving (1k cycles); bias via DVE add in f32 -> fp16
    AllToAll esT -> rcv[g*2+bl, j] = energy[2c+bl, g*128+j]  (4 KB)
    one PE transpose (16-identity) -> energy[j, jt, bl] fp16 in PSUM,
        scalar copy to SBUF
    scores[b, s] = sum_h energy[h, b] enc[b, h, s]     (PE, fp16, f32 PSUM
        accum; the two batches run concurrently in separate 32-col groups
        via tile_position derived from out partitions 0/32)
    probs = softmax over s (joint [33,*] ops: both batches per instruction)

Ring assignment (all measured): 6 of the 8 enc MB-sized transfers ride
the sync HWDGE ring and 2 the scalar HWDGE ring (one ring alone caps at
~337 GB/s; split, the pure stream hits the ~358 GB/s per-NC HBM
ceiling).  Every other data DMA also rides a HWDGE ring — one routine
SWDGE (gpsimd) load measured +3.2 us/rep of Q7 descriptor-emission cost.
Only the collective itself is issued from gpsimd.  The softmax + output
stores of rep N are emitted AFTER rep N+1's input loads (software
pipelining): the ACT sequencer both executes Exp and issues the
scalar-ring DMAs, so late-dependency work must sit behind the next rep's
loads or it stalls the input stream.  Scores accumulate into 4 per-chunk
PSUM banks drained to SBUF by DVE as each chunk completes.

Known residual: the AllToAll's serial latency is ~5.7 us idle but
~23 us under the full enc stream (SDMA packet round-robin + loaded-HBM
handshake costs), and NRT executes collectives strictly serially, so it
costs ~3 us/rep on top of the 24.5 us DMA floor (measured via a local
loopback A/B).  Shared-output AllGather, sub-mesh groups, gpsimd bounce
placement, and deeper pipelining were all measured and did not help.

`reps` exists only for benchmarking: the body is repeated (statically
unrolled — collectives cannot live inside a For_i hardware loop, and the
loop barrier costs ~3.4 us/rep anyway) inside one NEFF so steady-state HW
time per rep can be measured through the high-latency axon dispatch path.
Every rep is a self-contained emulation of one kernel() call: it re-loads
all inputs from DRAM and re-runs the full computation.  kernel() always
uses reps=1.

Measured (median-of-9 wall clock, marginal 64->1024 reps): ~28.5 us/rep
vs 33.4 us/rep for the replicated-W baseline under the same harness;
floor is ~24.5 us (enc stream at the HBM ceiling) + ~3 us collective.
"""

import numpy as np

S, B, H = 2048, 16, 1024
NCORES = 8
BL = B // NCORES  # 2 batches per core
P = 128           # partitions
HT = H // P       # 8 h-tiles
SCW = 512         # matmul moving-operand chunk (one PSUM bank of f32)
SC = S // SCW     # 4 seq chunks
ENC_BUFS = 12     # 12 MB of enc runway so the input stream never waits on
                  # the (energy-chain-gated) scores consumer
ENC_SCALAR = 1    # how many of the 8 enc h-tiles ride the scalar ring
SIM_PSUM_INIT = False  # CoreSim only: memset score PSUM so partitions 1-31
                       # (never written on HW, never read downstream) don't
                       # trip the uninitialized-memory check
NO_CC = False          # timing probe only: local loopback instead of AllToAll
CC_SINGLETON = False   # timing probe only: per-core singleton replica groups
CC_ALLGATHER_STATIC = False  # timing probe only: Shared AllGather + WRONG static row read
                       # (outputs wrong for cores != contribution layout)
CONSTS_BUFS = 2
DRAM_BUFS = 2
WG = 8             # W-shard group size (sub-mesh replica groups are not
                   # supported by NRT here, so this must stay 8)

_cached = {}


def _build_nc(reps=1):
    import concourse.bacc as bacc
    import concourse.bass as bass
    import concourse.tile as tile
    from concourse import mybir

    f16 = mybir.dt.float16
    f32 = mybir.dt.float32
    nc = bacc.Bacc("TRN2", target_bir_lowering=False, debug=False, num_devices=NCORES)

    JW = H // WG       # W rows (energy features) per core
    SUBT = JW // P     # 128-wide sub-tiles per slice
    GB = BL * WG       # batches per replica group

    # host-prepped layouts (see prep_in_maps), c = core id, r = c % WG,
    # g = c // WG:
    # enc_t[ht, p, bl, s] = enc[s, 2c+bl, ht*128+p]           fp16
    # wslt[p, it, jj]     = W[r*JW + jj, it*128 + p]          fp16
    # stT[p, it, k]       = state[g*GB + k, it*128 + p]       fp16
    # bias_bc[k, jj]      = bias[r*JW + jj]                   f32
    # id16                = eye(16)                           fp16
    enc_t = nc.dram_tensor("enc_t", [HT, P, BL, S], f16, kind="ExternalInput").ap()
    wslt = nc.dram_tensor("wslt", [P, HT, JW], f16, kind="ExternalInput").ap()
    stT = nc.dram_tensor("stT", [P, HT, GB], f16, kind="ExternalInput").ap()
    bias_bc = nc.dram_tensor("bias_bc", [GB, JW], f32, kind="ExternalInput").ap()
    id16 = nc.dram_tensor("id16", [B, B], f16, kind="ExternalInput").ap()
    probs = nc.dram_tensor("probs", [BL, S], f32, kind="ExternalOutput").ap()

    with tile.TileContext(nc) as tc:
        with (
            tc.tile_pool(name="consts", bufs=CONSTS_BUFS) as consts,
            tc.tile_pool(name="wpool", bufs=2) as wpool,
            tc.tile_pool(name="encpool", bufs=ENC_BUFS) as encpool,
            tc.tile_pool(name="dram", bufs=DRAM_BUFS, space="DRAM") as dram,
            tc.tile_pool(name="e_ps", bufs=2, space=bass.MemorySpace.PSUM) as e_pool,
            tc.tile_pool(name="t_ps", bufs=2, space=bass.MemorySpace.PSUM) as t_pool,
            tc.tile_pool(name="sc_ps", bufs=1, space=bass.MemorySpace.PSUM) as ps_pool,
            tc.tile_pool(name="spool", bufs=2) as spool,
        ):

            def emit_head():
                """Loads + energy chain + scores of one rep.  Returns the
                state the deferred softmax tail needs."""
                # --- input loads, all on HWDGE rings (a single SWDGE
                # gpsimd load measured +3.2 us/rep; Q7 descriptor emission
                # is far more expensive than the bytes)
                st = consts.tile([P, HT, GB], f16)
                nc.scalar.dma_start(out=st[:], in_=stT)
                ws = wpool.tile([P, HT, JW], f16)
                nc.scalar.dma_start(out=ws[:], in_=wslt)
                bb = consts.tile([GB, JW], f32)
                nc.scalar.dma_start(out=bb[:], in_=bias_bc)
                idt = consts.tile([B, B], f16)
                nc.scalar.dma_start(out=idt[:], in_=id16)

                ets = [None] * HT
                for ht in range(HT):
                    et = encpool.tile([P, BL, S], f16)
                    eng = nc.scalar if ht < ENC_SCALAR else nc.sync
                    eng.dma_start(out=et[:], in_=enc_t[ht])
                    ets[ht] = et

                # --- energy slice: esT[k, jj] for the group's batches
                esT_ps = e_pool.tile([GB, JW], f32)
                for it in range(HT):
                    nc.tensor.matmul(
                        esT_ps[:],
                        st[:, it, :],    # lhsT [i, b] (stationary)
                        ws[:, it, :],    # rhs  [i, j] (moving)
                        start=(it == 0),
                        stop=(it == HT - 1),
                    )
                esT = consts.tile([GB, JW], f16)
                nc.vector.tensor_tensor(
                    out=esT[:], in0=esT_ps[:], in1=bb[:], op=mybir.AluOpType.add
                )

                # --- AllToAll: chunk d (batches 2d:2d+2) of my j-slice ->
                # core d; only the collective itself runs on gpsimd — every
                # SWDGE dma_start measured ~1-3 us of Q7 descriptor-emission
                # cost, so all data DMAs ride the HWDGE rings
                # ib row 2d+bl = chunk d (batches 2d:2d+2 of my j-slice)
                ib = dram.tile([B, P], f16)
                nc.scalar.dma_start(out=ib[:], in_=esT[:])
                rcv = dram.tile([B, P], f16)
                if NO_CC:
                    nc.scalar.dma_start(out=rcv[:], in_=ib[:])
                    g_src = rcv[:]
                elif CC_ALLGATHER_STATIC:
                    rcv_g = dram.tile([NCORES, B, P], f16, addr_space="Shared")
                    nc.gpsimd.collective_compute(
                        "AllGather",
                        mybir.AluOpType.bypass,
                        replica_groups=[list(range(NCORES))],
                        ins=[ib.opt()],
                        outs=[rcv_g.opt()],
                    )
                    # WRONG rows (0:2 instead of 2c:2c+2) - timing only
                    g_src = rcv_g[:, 0:BL, :]
                else:
                    groups = ([[c] for c in range(NCORES)] if CC_SINGLETON
                              else [list(range(g * WG, (g + 1) * WG))
                                    for g in range(NCORES // WG)])
                    nc.gpsimd.collective_compute(
                        "AllToAll",
                        mybir.AluOpType.bypass,
                        replica_groups=groups,
                        ins=[ib.opt()],
                        outs=[rcv.opt()],
                    )
                    g_src = rcv[:]
                # rcv[src*BL+bl, jj] = energy[my batch bl, src*128+jj]
                g_sb = consts.tile([B, P], f16)
                nc.scalar.dma_start(out=g_sb[:], in_=g_src)

                # --- transpose to energy[j, (jt, bl)] for the scores lhsT
                gT_ps = t_pool.tile([P, B], f16)
                nc.tensor.transpose(gT_ps[:], g_sb[:], idt[:])
                esb = consts.tile([P, HT, BL], f16)
                nc.scalar.activation(
                    out=esb[:], in_=gT_ps[:],
                    func=mybir.ActivationFunctionType.Identity,
                    bias=0.0, scale=1.0,
                )

                # --- scores psum: b=0 on partition 0, b=1 on partition 32;
                # the two batches run concurrently in separate 32-col
                # groups.  One PSUM bank ([33, SCW]) per seq chunk; DVE
                # drains each chunk to SBUF (and takes its max) as soon as
                # it completes so the banks recycle without waiting for the
                # softmax, which is emitted a rep later (see emit_tail).
                pscs = [
                    ps_pool.tile([33, SCW], f32, name=f"psc{sc}")
                    for sc in range(SC)
                ]
                if SIM_PSUM_INIT:
                    for psc in pscs:
                        nc.vector.memset(psc[:], 0.0)
                sc_sb = spool.tile([33, S], f32)
                nmaxs = spool.tile([33, SC], f32)
                for ht in range(HT):
                    et = ets[ht]
                    for sc in range(SC):
                        for b in range(BL):
                            nc.tensor.matmul(
                                pscs[sc][32 * b:32 * b + 1, :],
                                esb[:, ht, b:b + 1],                  # lhsT [h, 1]
                                et[:, b, sc * SCW:(sc + 1) * SCW],    # rhs [h, s]
                                start=(ht == 0),
                                stop=(ht == HT - 1),
                                tile_position=(0, 32 * b),
                            )
                        if ht == HT - 1:
                            nc.vector.reduce_max(
                                nmaxs[:, sc:sc + 1], pscs[sc][:],
                                axis=mybir.AxisListType.X, negate=True,
                            )
                            nc.vector.tensor_copy(
                                out=sc_sb[:, sc * SCW:(sc + 1) * SCW],
                                in_=pscs[sc][:],
                            )
                return sc_sb, nmaxs

            def emit_tail(sc_sb, nmaxs):
                """Softmax + store of the PREVIOUS rep.  Emitted after the
                next rep's loads so the in-order ACT sequencer (which both
                executes Exp and issues the scalar-ring DMAs) never makes
                the input stream wait on late compute."""
                prob_sb = spool.tile([33, S], f32)
                nmax = spool.tile([33, 1], f32)
                ssums = spool.tile([33, SC], f32)
                ssum = spool.tile([33, 1], f32)
                rinv = spool.tile([33, 1], f32)
                nc.vector.tensor_reduce(
                    out=nmax[:], in_=nmaxs[:], op=mybir.AluOpType.min,
                    axis=mybir.AxisListType.X,
                )
                for sc in range(SC):
                    nc.scalar.activation(
                        out=prob_sb[:, sc * SCW:(sc + 1) * SCW],
                        in_=sc_sb[:, sc * SCW:(sc + 1) * SCW],
                        func=mybir.ActivationFunctionType.Exp,
                        bias=nmax[:],
                        scale=1.0,
                        accum_out=ssums[:, sc:sc + 1],
                    )
                nc.vector.tensor_reduce(
                    out=ssum[:], in_=ssums[:], op=mybir.AluOpType.add,
                    axis=mybir.AxisListType.X,
                )
                nc.vector.reciprocal(rinv[:], ssum[:])
                nc.vector.tensor_scalar_mul(
                    out=prob_sb[:], in0=prob_sb[:], scalar1=rinv[:]
                )
                for b in range(BL):
                    nc.scalar.dma_start(
                        out=probs[b:b + 1, :], in_=prob_sb[32 * b:32 * b + 1, :]
                    )

            pending = None
            for _rep in range(reps):
                state = emit_head()
                if pending is not None:
                    emit_tail(*pending)
                pending = state
            emit_tail(*pending)

    nc.compile()
    return nc


def get_nc(reps=1, dynamic=False):
    # `dynamic` is accepted for interface compat but ignored: collectives
    # cannot live inside a For_i hardware loop, so reps are always
    # statically unrolled.
    key = ("nc", reps, ENC_BUFS, ENC_SCALAR, SIM_PSUM_INIT, NO_CC,
           CC_SINGLETON, CC_ALLGATHER_STATIC, CONSTS_BUFS, DRAM_BUFS, WG)
    if key not in _cached:
        _cached[key] = _build_nc(reps)
    return _cached[key]


def prep_in_maps(encoder_output, last_decoder_state, W, b):
    JW = H // WG
    GB = BL * WG
    enc16 = np.asarray(encoder_output, dtype=np.float32).astype(np.float16)  # [S,B,H]
    state = np.asarray(last_decoder_state, dtype=np.float32)[0, 0]           # [B,H]
    W16 = np.asarray(W, dtype=np.float32).astype(np.float16)
    bias = np.asarray(b, dtype=np.float32)
    st16 = state.astype(np.float16)
    id16 = np.eye(B, dtype=np.float16)
    in_maps = []
    for c in range(NCORES):
        r, g = c % WG, c // WG
        b0 = BL * c
        ec = enc16[:, b0:b0 + BL, :]                                         # [S,BL,H]
        enc_t = np.ascontiguousarray(ec.transpose(2, 1, 0)).reshape(HT, P, BL, S)
        wsl = W16[r * JW:(r + 1) * JW, :]                                    # [jj, H]
        wslt = np.ascontiguousarray(wsl.reshape(JW, HT, P).transpose(2, 1, 0))
        stT = np.ascontiguousarray(
            st16[g * GB:(g + 1) * GB].reshape(GB, HT, P).transpose(2, 1, 0)
        )  # [p, it, k]
        bias_bc = np.ascontiguousarray(
            np.broadcast_to(bias[r * JW:(r + 1) * JW], (GB, JW))
        )
        in_maps.append({"enc_t": enc_t, "wslt": wslt, "stT": stT,
                        "bias_bc": bias_bc, "id16": id16})
    return in_maps


def assemble(results):
    out = np.empty((S, B), np.float32)
    for c in range(NCORES):
        out[:, BL * c:BL * (c + 1)] = results[c]["probs"].T
    return out[None, None]


def kernel(encoder_output, last_decoder_state, W, b):
    from concourse.bass_utils import run_bass_kernel_spmd

    nc = get_nc()
    in_maps = prep_in_maps(encoder_output, last_decoder_state, W, b)
    res = run_bass_kernel_spmd(nc, in_maps, core_ids=list(range(NCORES)))
    return assemble(res.results)
